# revision 7
# baseline (speedup 1.0000x reference)
"""Trainium2 Bass kernel for nn_Attention_58652073394851.

out[n] = sum_s alpha_s[n] * Z_s[n],  alpha_s = softmax_N(tanh(Z_s @ W_s.T + b_s.T) @ q)

Strategy (8 NeuronCores, data-parallel over N):
  - Host shards N=100000 into 8 chunks of 12500, zero-pads each to 12544 rows
    (98 tiles of 128), and also passes a host-transposed copy Z.T per stream so
    the score matmul can stream Z with D on partitions (no on-chip transpose).
  - Phase 1 (per core): h.T = tanh(W Z.T + b) via PE matmuls (K=128 x2 halves),
    s-columns via per-tile matmul h @ q -> scores [128, 98] per stream.
  - exp(s) without max-subtraction (|s| <= ||q||_1 ~ 6.5, no overflow in f32);
    row sums via ACT accum_out; partition sum via PE matmul with ones.
  - One AllReduce(add) of the 3 per-stream local sums (12 bytes).
  - Phase 2: out_tile = sum_s alpha_s[:,t] * Z_s[t]  (ACT mul + 2 DVE
    scalar_tensor_tensor fused mul-adds), alpha = exp(s) * (1/S_global).
"""

import numpy as np

N_TOTAL = 100000
D = 256
H = 64
NCORES = 8
PN = N_TOTAL // NCORES          # 12500 real rows per core
TILES = 98                      # padded tiles of 128 rows
ROWS = TILES * 128              # 12544 padded rows per core
PAD_TILE = TILES - 1            # tile containing padding
PAD_PART = PN - PAD_TILE * 128  # first padded partition in that tile (84)

# chunking: phase 1/2 process 8 tiles (1024 rows) per DMA
CHUNKS = [8] * 12 + [2]

_CACHE = {}


def _build_program(collective=True):
    import concourse.bacc as bacc
    import concourse.mybir as mybir
    from concourse.tile import TileContext
    from contextlib import ExitStack

    f32 = mybir.dt.float32
    AF = mybir.ActivationFunctionType
    ALU = mybir.AluOpType

    nc = bacc.Bacc(None, target_bir_lowering=False, num_devices=NCORES)

    zt_d = [nc.dram_tensor(f"zt_{s}", [D, ROWS], f32, kind="ExternalInput")
            for s in "TCF"]
    zn_d = [nc.dram_tensor(f"zn_{s}", [ROWS, D], f32, kind="ExternalInput")
            for s in "TCF"]
    wt_d = nc.dram_tensor("wt", [128, 2, 3, H], f32, kind="ExternalInput")
    bq_d = nc.dram_tensor("bq", [H, 4], f32, kind="ExternalInput")
    # per-stream sum of exp(score) over this core's PAD rows (host-computed:
    # pad rows have Z=0 -> score = tanh(b_s) . q, identical for all pads)
    padc_d = nc.dram_tensor("padc", [1, 3], f32, kind="ExternalInput")
    out_d = nc.dram_tensor("out", [ROWS, D], f32, kind="ExternalOutput")

    zn_v = [z.rearrange("(t p) d -> p t d", p=128) for z in zn_d]
    out_v = out_d.rearrange("(t p) d -> p t d", p=128)

    with TileContext(nc) as tc, ExitStack() as ctx:
        const = ctx.enter_context(tc.tile_pool(name="const", bufs=1))
        persist = ctx.enter_context(tc.tile_pool(name="persist", bufs=1))
        io1 = ctx.enter_context(tc.tile_pool(name="io1", bufs=3))
        work1 = ctx.enter_context(tc.tile_pool(name="work1", bufs=4))
        io2 = ctx.enter_context(tc.tile_pool(name="io2", bufs=3))
        work2 = ctx.enter_context(tc.tile_pool(name="work2", bufs=3))
        ps_h = ctx.enter_context(tc.tile_pool(name="ps_h", bufs=2, space="PSUM"))
        ps_s = ctx.enter_context(tc.tile_pool(name="ps_s", bufs=2, space="PSUM"))
        ps_m = ctx.enter_context(tc.tile_pool(name="ps_m", bufs=1, space="PSUM"))
        dram = ctx.enter_context(tc.tile_pool(name="dram", bufs=1, space="DRAM"))

        wt_sb = const.tile([128, 2, 3, H], f32)
        nc.sync.dma_start(wt_sb[:], wt_d[:])
        bq_sb = const.tile([H, 4], f32)
        nc.sync.dma_start(bq_sb[:], bq_d[:])
        ones_col = const.tile([128, 1], f32)
        nc.vector.memset(ones_col[:], 1.0)
        ones_row = const.tile([1, 128], f32)
        nc.vector.memset(ones_row[:], 1.0)
        zero128 = const.tile([128, 1], f32)
        nc.vector.memset(zero128[:], 0.0)

        score = [persist.tile([128, TILES], f32, tag=f"score{s}", name=f"score{s}")
                 for s in range(3)]
        alpha = [persist.tile([128, TILES], f32, tag=f"alpha{s}", name=f"alpha{s}")
                 for s in range(3)]
        rowsum = persist.tile([128, 3], f32, tag="rowsum")

        # ---------------- phase 1: scores ----------------
        for s in range(3):
            t0 = 0
            for ct in CHUNKS:
                ncols = ct * 128
                zt0 = io1.tile([128, 1024], f32, tag="zt0")
                zt1 = io1.tile([128, 1024], f32, tag="zt1")
                nc.sync.dma_start(zt0[:, 0:ncols],
                                  zt_d[s][0:128, t0 * 128: t0 * 128 + ncols])
                nc.sync.dma_start(zt1[:, 0:ncols],
                                  zt_d[s][128:256, t0 * 128: t0 * 128 + ncols])
                sp = ps_s.tile([128, 8], f32, tag="sp")
                for g0 in range(0, ct, 4):
                    gt = min(4, ct - g0)
                    gc = gt * 128
                    c0 = g0 * 128
                    hp = ps_h.tile([H, 512], f32, tag="hp")
                    nc.tensor.matmul(hp[:, 0:gc], wt_sb[:, 0, s, :],
                                     zt0[:, c0:c0 + gc], start=True, stop=False)
                    nc.tensor.matmul(hp[:, 0:gc], wt_sb[:, 1, s, :],
                                     zt1[:, c0:c0 + gc], start=False, stop=True)
                    ht = work1.tile([H, 512], f32, tag="ht")
                    nc.scalar.activation(ht[:, 0:gc], hp[:, 0:gc], AF.Tanh,
                                         bias=bq_sb[:, s:s + 1])
                    for j in range(gt):
                        nc.tensor.matmul(sp[:, g0 + j:g0 + j + 1],
                                         ht[:, j * 128:(j + 1) * 128],
                                         bq_sb[:, 3:4])
                nc.vector.tensor_copy(score[s][:, t0:t0 + ct], sp[:, 0:ct])
                t0 += ct

        # exp + per-partition row sums
        for s in range(3):
            nc.scalar.activation(alpha[s][:], score[s][:], AF.Exp,
                                 bias=zero128[:], accum_out=rowsum[:, s:s + 1])

        # local sums [1,3] via PE partition-reduce; subtract the padding
        # rows' contribution so they don't enter the softmax denominator
        padc_sb = const.tile([1, 3], f32)
        nc.sync.dma_start(padc_sb[:], padc_d[:])
        sl_ps = ps_m.tile([1, 3], f32, tag="sl")
        nc.tensor.matmul(sl_ps[:], ones_col[:], rowsum[:])
        sl_sb = persist.tile([1, 3], f32, tag="slsb")
        nc.vector.tensor_tensor(sl_sb[:], sl_ps[:], padc_sb[:],
                                op=ALU.subtract)

        # ---------------- AllReduce of sums ----------------
        sg_sb = persist.tile([1, 3], f32, tag="sgsb")
        if collective:
            cc_in = dram.tile([1, 3], f32, tag="ccin")
            cc_out = dram.tile([1, 3], f32, tag="ccout")
            nc.gpsimd.dma_start(cc_in[:], sl_sb[:])
            nc.gpsimd.collective_compute(
                "AllReduce", ALU.add,
                replica_groups=[list(range(NCORES))],
                ins=[cc_in[:].opt()],
                outs=[cc_out[:].opt()],
            )
            nc.gpsimd.dma_start(sg_sb[:], cc_out[:])
        else:
            # single-core timeline-sim variant: pretend local sum is global
            nc.vector.tensor_copy(sg_sb[:], sl_sb[:])

        inv_sb = persist.tile([1, 3], f32, tag="invsb")
        nc.vector.reciprocal(inv_sb[:], sg_sb[:])
        bc_ps = ps_m.tile([128, 3], f32, tag="bc")
        nc.tensor.matmul(bc_ps[:], ones_row[:], inv_sb[:])
        invb = persist.tile([128, 3], f32, tag="invb")
        nc.vector.tensor_copy(invb[:], bc_ps[:])

        # alpha = exp(s) / S_global   (in place)
        for s in range(3):
            nc.vector.tensor_scalar_mul(alpha[s][:], alpha[s][:],
                                        invb[:, s:s + 1])

        # ---------------- phase 2: weighted sum ----------------
        t0 = 0
        for ct in CHUNKS:
            zn = [io2.tile([128, 8, D], f32, tag=f"zn{s}", name=f"zn{s}")
                  for s in range(3)]
            for s in range(3):
                nc.sync.dma_start(zn[s][:, 0:ct, :], zn_v[s][:, t0:t0 + ct, :])
            ob = work2.tile([128, 8, D], f32, tag="ob")
            for j in range(ct):
                t = t0 + j
                nc.scalar.activation(ob[:, j, :], zn[0][:, j, :], AF.Copy,
                                     scale=alpha[0][:, t:t + 1])
                nc.vector.scalar_tensor_tensor(
                    ob[:, j, :], zn[1][:, j, :], alpha[1][:, t:t + 1],
                    ob[:, j, :], op0=ALU.mult, op1=ALU.add)
                nc.vector.scalar_tensor_tensor(
                    ob[:, j, :], zn[2][:, j, :], alpha[2][:, t:t + 1],
                    ob[:, j, :], op0=ALU.mult, op1=ALU.add)
            nc.sync.dma_start(out_v[:, t0:t0 + ct, :], ob[:, 0:ct, :])
            t0 += ct

    nc.compile()
    return nc


def _get_program():
    if "nc" not in _CACHE:
        _CACHE["nc"] = _build_program()
    return _CACHE["nc"]


def _prep_in_maps(inputs):
    f32 = np.float32
    Zs = [np.ascontiguousarray(np.asarray(inputs[f"Z_{s}"], dtype=f32))
          for s in "TCF"]
    Ws = [np.asarray(inputs[f"W_{s}"], dtype=f32) for s in "TCF"]
    bs = [np.asarray(inputs[f"b_{s}"], dtype=f32) for s in "TCF"]
    q = np.asarray(inputs["q"], dtype=f32)

    # wt_pack[p, h, s, j] = W_s[j, h*128 + p]
    wt = np.stack([W.T.reshape(2, 128, H) for W in Ws])       # [3, 2, 128, 64]
    wt_pack = np.ascontiguousarray(wt.transpose(2, 1, 0, 3))  # [128, 2, 3, 64]
    bq = np.ascontiguousarray(np.concatenate(bs + [q], axis=1))  # [64, 4]
    # padding rows have Z=0 -> score = tanh(b_s).q; their exp contribution
    # is removed from the local softmax denominator on-device
    padc = np.array([[(ROWS - PN) * np.exp(np.tanh(b[:, 0]) @ q[:, 0])
                      for b in bs]], dtype=f32)

    in_maps = []
    for i in range(NCORES):
        m = {"wt": wt_pack, "bq": bq, "padc": padc}
        for s, name in enumerate("TCF"):
            zp = np.zeros((ROWS, D), dtype=f32)
            zp[:PN] = Zs[s][i * PN:(i + 1) * PN]
            m[f"zn_{name}"] = zp
            m[f"zt_{name}"] = np.ascontiguousarray(zp.T)
        in_maps.append(m)
    return in_maps


LAST_RESULTS = None


def kernel(**inputs) -> np.ndarray:
    global LAST_RESULTS
    from concourse.bass_utils import run_bass_kernel_spmd

    nc = _get_program()
    in_maps = _prep_in_maps(inputs)
    res = run_bass_kernel_spmd(nc, in_maps, core_ids=list(range(NCORES)))
    LAST_RESULTS = res
    out = np.concatenate([res.results[i]["out"][:PN] for i in range(NCORES)],
                         axis=0)
    return out


if __name__ == "__main__":
    rng = np.random.default_rng(0)
    ins = {
        "Z_T": rng.standard_normal((N_TOTAL, D), dtype=np.float32),
        "Z_C": rng.standard_normal((N_TOTAL, D), dtype=np.float32),
        "Z_F": rng.standard_normal((N_TOTAL, D), dtype=np.float32),
        "W_T": rng.standard_normal((H, D), dtype=np.float32) / 8,
        "b_T": rng.standard_normal((H, 1), dtype=np.float32) / 8,
        "W_C": rng.standard_normal((H, D), dtype=np.float32) / 8,
        "b_C": rng.standard_normal((H, 1), dtype=np.float32) / 8,
        "W_F": rng.standard_normal((H, D), dtype=np.float32) / 8,
        "b_F": rng.standard_normal((H, 1), dtype=np.float32) / 8,
        "q": rng.standard_normal((H, 1), dtype=np.float32) / 8,
    }
    out = kernel(**ins)
    print(out.shape, out.dtype)
